# revision 2
# baseline (speedup 1.0000x reference)
import sys

sys.path.insert(0, "/opt/trn_rl_repo")

import numpy as np
import ml_dtypes

import concourse.bass as bass
import concourse.tile as tile
from concourse import bacc, mybir
from concourse.bass_utils import run_bass_kernel_spmd

# ---- problem constants (hardcoded per contract) ----
B, N, F = 8, 512, 16
D, PP = 150, 26
IMG = 128
NB = 4                  # render batches of 128 z-sorted emitters
NCHUNK = 18             # 128-row K chunks per batch (2 z-slabs of 64)
RTB = 3                 # rt chunks per DMA
RN = PP * PP            # 676
W184 = 184              # dram row stride (elements)
LEAD = 4                # leading zero rows per partition
TAIL = 3                # trailing zero rows
JROWS = LEAD + NB * PP + TAIL   # 111
SEG = JROWS * W184      # 20424 elements per partition
NBLK = 16
NQ = 32                 # main quads (4 canvas rows each)
QI = NQ + NBLK          # + one spill quad per block = 48 gather chunks
CH = 3 * W184 + 128     # 680 elements per gather chunk (4 rows)
XW = 128                # canvas x window [26,154)

_compiled = None


def _build_bass(debug=False):
    nc = bacc.Bacc()
    f32 = mybir.dt.float32
    bf16 = mybir.dt.bfloat16
    i32 = mybir.dt.int32

    rlhs_d = nc.declare_dram_parameter(
        "rlhs", [NB, 128, NCHUNK * 128], bf16, isOutput=False
    )
    rslab_d = nc.declare_dram_parameter(
        "rslab", [NB, NCHUNK // RTB, 128, RTB * RN], bf16, isOutput=False
    )
    idx_d = nc.declare_dram_parameter("idx", [128, QI], i32, isOutput=False)
    rowlhs_d = nc.declare_dram_parameter(
        "rowlhs", [128, QI * 4 * 128], bf16, isOutput=False
    )
    out_d = nc.declare_dram_parameter("out", [128, NBLK * XW], bf16, isOutput=True)
    dbg = {}
    if debug:
        dbg["rend"] = nc.declare_dram_parameter(
            "dbg_rend", [128, SEG], bf16, isOutput=True
        )
        dbg["strip"] = nc.declare_dram_parameter(
            "dbg_strip", [128, QI * CH], bf16, isOutput=True
        )

    with tile.TileContext(nc) as tc:
        with (
            tc.tile_pool(name="big", bufs=1) as big_pool,
            tc.tile_pool(name="lt", bufs=2) as lt_pool,
            tc.tile_pool(name="rt", bufs=4) as rt_pool,
            tc.tile_pool(name="psr", bufs=2, space="PSUM") as psr_pool,
            tc.tile_pool(name="psc", bufs=4, space="PSUM") as psc_pool,
            tc.tile_pool(name="dram", bufs=1, space="DRAM") as dram_pool,
        ):
            idx_t = big_pool.tile([128, QI], i32, tag="idx")
            nc.scalar.dma_start(idx_t[:], idx_d[:])

            rend = big_pool.tile([128, SEG], bf16, tag="rend")
            nc.vector.memset(rend[:].bitcast(i32), 0)

            rows_d = dram_pool.tile([128, SEG], bf16, tag="rows")
            rowlhs_t = big_pool.tile([128, QI * 4 * 128], bf16, tag="rowlhs")

            for m in range(NB):
                lt = lt_pool.tile([128, NCHUNK * 128], bf16, tag="lt")
                nc.sync.dma_start(lt[:], rlhs_d[m])
                ps = psr_pool.tile([128, RN], f32, tag="ps")
                for cb in range(NCHUNK // RTB):
                    rt = rt_pool.tile([128, RTB * RN], bf16, tag="rt")
                    (nc.sync if cb % 2 == 0 else nc.scalar).dma_start(
                        rt[:], rslab_d[m, cb]
                    )
                    for cc in range(RTB):
                        c = cb * RTB + cc
                        for n0, n1 in ((0, 512), (512, RN)):
                            nc.tensor.matmul(
                                ps[:, n0:n1],
                                lhsT=lt[:, c * 128 : (c + 1) * 128],
                                rhs=rt[:, cc * RN + n0 : cc * RN + n1],
                                start=(c == 0),
                                stop=(c == NCHUNK - 1),
                            )
                if m == 1:
                    # big table needed only by row-MM; load mid-render
                    nc.scalar.dma_start(
                        rowlhs_t[:, : QI * 2 * 128], rowlhs_d[:, : QI * 2 * 128]
                    )
                    nc.scalar.dma_start(
                        rowlhs_t[:, QI * 2 * 128 :], rowlhs_d[:, QI * 2 * 128 :]
                    )
                nc.scalar.copy(
                    out=rend[:].rearrange("p (j w) -> p j w", j=JROWS)[
                        :, m * 26 + LEAD : m * 26 + LEAD + 26, 0:26
                    ],
                    in_=ps[:].rearrange("p (h w) -> p h w", h=26),
                )
                nc.sync.dma_start(
                    rows_d[:, (m * 26 + LEAD) * W184 : (m * 26 + LEAD + 26) * W184],
                    rend[:, (m * 26 + LEAD) * W184 : (m * 26 + LEAD + 26) * W184],
                )
            # zero lead + tail rows
            nc.sync.dma_start(rows_d[:, 0 : LEAD * W184], rend[:, 0 : LEAD * W184])
            nc.sync.dma_start(
                rows_d[:, (JROWS - TAIL) * W184 :], rend[:, (JROWS - TAIL) * W184 :]
            )

            strip = big_pool.tile([128, QI * CH], bf16, tag="strip")
            strip3 = strip[:].rearrange("p (q x) -> p q x", q=QI)
            rows_flat = rows_d[:].rearrange("p (n o) -> (p n) o", o=1)
            for q in range(QI):
                nc.gpsimd.indirect_dma_start(
                    out=strip3[:, q, :],
                    out_offset=None,
                    in_=rows_flat,
                    in_offset=bass.IndirectOffsetOnAxis(
                        ap=idx_t[:, q : q + 1], axis=0
                    ),
                )

            if debug:
                nc.sync.dma_start(dbg["rend"][:], rend[:])
                nc.sync.dma_start(dbg["strip"][:], strip[:])

            canvas = big_pool.tile([128, NBLK * XW], bf16, tag="canvas")
            rowlhs3 = rowlhs_t[:].rearrange("p (y m) -> p y m", y=QI * 4)
            for blk in range(NBLK):
                pc = psc_pool.tile([128, XW], f32, tag="pc")
                qs = [2 * blk, 2 * blk + 1, NQ + blk]
                for j, q in enumerate(qs):
                    for k in range(4):
                        nc.tensor.matmul(
                            pc[:],
                            lhsT=rowlhs3[:, q * 4 + k, :],
                            rhs=strip3[:, q, k * W184 : k * W184 + XW],
                            start=(j == 0 and k == 0),
                            stop=(j == 2 and k == 3),
                        )
                nc.scalar.copy(
                    out=canvas[:, blk * XW : (blk + 1) * XW], in_=pc[:]
                )
            nc.sync.dma_start(out_d[:, 0 : 8 * XW], canvas[:, 0 : 8 * XW])
            nc.scalar.dma_start(out_d[:, 8 * XW :], canvas[:, 8 * XW :])
    if not nc.is_finalized():
        nc.finalize()
    return nc


def _host_prep(xyz, n_photons, coeffs, inv_voxel_size, psf_center):
    u = xyz * inv_voxel_size
    u = u.copy()
    u[..., :2] -= psf_center[:2]
    u[..., 2] += psf_center[2]
    u_floor = np.floor(u)
    frac = u - u_floor
    ui = u_floor.astype(np.int32)
    x_idx = ui[..., 0] + PP
    y_idx = ui[..., 1] + PP
    z_idx = ui[..., 2]
    frac[..., :2] = 1.0 - frac[..., :2]

    p4 = frac[..., None] ** np.arange(4, dtype=np.float32)
    vx, vy, vz = p4[..., 0, :], p4[..., 1, :], p4[..., 2, :]
    series = (
        vz[..., :, None, None] * vx[..., None, :, None] * vy[..., None, None, :]
    ).reshape(B, N, 64)

    slab = np.ascontiguousarray(coeffs.transpose(0, 3, 1, 2).reshape(D, 64, RN))

    in_maps = []
    for b in range(B):
        in_maps.append(
            _prep_one(x_idx[b], y_idx[b], z_idx[b], series[b], n_photons[b], slab)
        )
    return in_maps


def _prep_one(x_idx, y_idx, z_idx, series, photons, slab):
    order = np.argsort(z_idx, kind="stable")
    pos = np.empty(N, dtype=np.int64)
    pos[order] = np.arange(N)

    rlhs = np.zeros((NB, NCHUNK, 128, 128), dtype=np.float32)
    rslab = np.zeros((NB, NCHUNK, 128, RN), dtype=ml_dtypes.bfloat16)
    for m in range(NB):
        es = order[m * 128 : (m + 1) * 128]
        zlist = np.unique(z_idx[es])
        assert len(zlist) <= 2 * NCHUNK
        zpos = {z: i for i, z in enumerate(zlist)}
        for i, z in enumerate(zlist):
            rslab[m, i // 2, 64 * (i % 2) : 64 * (i % 2) + 64, :] = slab[z]
        for col, e in enumerate(es):
            zi = zpos[z_idx[e]]
            rlhs[m, zi // 2, 64 * (zi % 2) : 64 * (zi % 2) + 64, col] = series[e]

    # quad packing: piece = (emitter, quad q) where patch covers rows in
    # [4q, 4q+4); overflow pieces go to the block's spill quad (NQ + blk)
    idx = np.full((128, QI), 26, dtype=np.int32)
    rowlhs = np.zeros((128, QI * 4, 128), dtype=np.float32)
    fill = np.zeros(QI, dtype=np.int64)

    def emit(cell_q, win_q, e):
        """place piece of emitter e (window = quad win_q rows) into gather
        cell column cell_q at the next free slot"""
        s = fill[cell_q]
        assert s < 128, f"cell overflow q={cell_q}"
        fill[cell_q] += 1
        r0 = 4 * win_q + 26 - y_idx[e]  # patch row at window row 0 (may be <0)
        p = pos[e] % 128
        jb = (pos[e] // 128) * 26 + LEAD
        idx[s, cell_q] = p * SEG + (jb + r0) * W184 + 26 - x_idx[e]
        for k in range(4):
            Y = 4 * win_q + k
            r = r0 + k
            if 0 <= r < 26 and 0 <= Y < IMG:
                col0 = (Y % 8) * 16
                rowlhs[s, cell_q * 4 + k, col0 : col0 + 16] = photons[e]

    for q in range(NQ):
        es = np.where((y_idx > 4 * q) & (y_idx <= 4 * q + 29))[0]
        for i, e in enumerate(es):
            if i < 128:
                emit(q, q, e)
            else:
                emit(NQ + q // 2, q, e)

    # partition-contiguous DRAM layouts (128 descriptors per DMA)
    rlhs_t = np.ascontiguousarray(rlhs.transpose(0, 2, 1, 3)).reshape(
        NB, 128, NCHUNK * 128
    )
    rslab_t = np.ascontiguousarray(
        rslab.reshape(NB, NCHUNK // RTB, RTB, 128, RN).transpose(0, 1, 3, 2, 4)
    ).reshape(NB, NCHUNK // RTB, 128, RTB * RN)
    return {
        "rlhs": rlhs_t.astype(ml_dtypes.bfloat16),
        "rslab": rslab_t,
        "idx": idx,
        "rowlhs": rowlhs.astype(ml_dtypes.bfloat16).reshape(128, QI * 4 * 128),
    }


def kernel(xyz, n_photons, coeffs, inv_voxel_size, psf_center, img_size):
    global _compiled
    xyz = np.asarray(xyz, dtype=np.float32)
    n_photons = np.asarray(n_photons, dtype=np.float32)
    coeffs = np.asarray(coeffs, dtype=np.float32)
    inv_voxel_size = np.asarray(inv_voxel_size, dtype=np.float32)
    psf_center = np.asarray(psf_center, dtype=np.float32)

    in_maps = _host_prep(xyz, n_photons, coeffs, inv_voxel_size, psf_center)

    if _compiled is None:
        _compiled = _build_bass()
    nc = _compiled

    res = run_bass_kernel_spmd(nc, in_maps, core_ids=list(range(B)))
    outs = []
    for b in range(B):
        c4 = res.results[b]["out"].astype(np.float32).reshape(8, 16, NBLK, XW)
        outs.append(c4.transpose(1, 2, 0, 3).reshape(F, IMG, IMG))
    return np.stack(outs, axis=0)
